# revision 6
# baseline (speedup 1.0000x reference)
"""Trainium2 Bass kernel for AttentionalAggregation-style GNN pooling.

reference math:
    enc  = relu(lane_encoding @ W.T + b)            # [M=400000, 512]
    maxp = segment_max(enc, seg)                    # [N=25000, 512], 16 lanes/group
    avgp = segment_mean(enc, seg)                   # [N=25000, 512]
    out  = concat([maxp, avgp], axis=1)             # [N, 2*512]

Strategy (8 NeuronCores, data-parallel over lanes; each core owns whole groups):
  - Host pre-transposes x -> XT [128, M] bf16 so the contraction dim is the
    SBUF partition dim for both matmul operands. Single-pass bf16 matmul
    (~3e-3 rel err, gate is 2e-2) -> PE time is 1/3 of a compensated bf16x3.
  - The wall is elementwise PSUM consumption + pooling. Spread across engines
    with two per-span strategies + routing knobs:
      * A-spans: ACT does relu(u+b) -> r bf16 in one pass; pooling is a
        radix-2 tensor_tensor tree in bf16 (DVE 2x_1p packed mode beats
        TensorReduce 2:1; TensorReduce has no fast modes). Trees rotate
        DVE/GpSimd by knob.
      * C-spans: ACT relus only the first half-group (strided [p,g,0:8]
        view of PSUM); custom DVE uops (registered at import into the
        concourse custom-DVE table) fuse the second half's relu+bias into
        tree level 1 (one PSUM operand per instruction — NCC_IBVF027
        allows only one):
           RELU_PLUS: out = relu(a + bias) + t0
           RELU_MAXX: out = max(relu(a + bias), t0)
        This halves ACT work per span at ~1.1x DVE cost.
  - No on-device epilogues: max pool is exact (relu is monotone, bias inside),
    sum pool is divided by 16 on the host during reassembly.
  - Pooled outputs stay transposed ([512, G] fp32); host reassembles.
"""
import sys

sys.path.insert(0, "/opt/trn_rl_repo")

import numpy as np
import ml_dtypes

import concourse.bass as bass
import concourse.bacc as bacc
import concourse.tile as tile
from concourse import mybir
from concourse.bass_utils import run_bass_kernel_spmd

import concourse.dve_ops as _dve_ops
from concourse.dve_spec import Spec, Src0, Src1, C0, relu, maxx, lower
from concourse.dve_spec import _has_src1
from concourse.dve_uop import DveOpSpec

N_CORES = 8
IN_DIM = 128
OUT_DIM = 512
N_OBS = 25000
M_LANES = 400000
GS = 16                       # lanes per group
M_C = M_LANES // N_CORES      # 50000 lanes per core
G_C = N_OBS // N_CORES        # 3125 groups per core
N_CHUNK = OUT_DIM // 128      # 4 outdim chunks
BLK = 2048                    # lanes per DMA/compute block (4 psum banks)

# --- load-balance knobs (tuned against the ntff profile) -------------------
# NOTE: the Pool (gpsimd) engine only implements add/mult tensor_tensor —
# no max op, no free-dim reduce — so max trees are DVE-only and gpsimd
# offload applies to SUM trees exclusively.
C_EIGHTHS = 2      # C-spans (half-ACT + fused DVE L1): span % 8 < this
GPS_TENTHS = 6     # fraction (tenths) of SUM trees routed to gpsimd

_compiled = {}


def _register_op(name: str, body, ref):
    """Register a custom DVE op into the concourse table at runtime.

    Follows the documented add-an-op flow (dve_ops.py docstring) without
    editing the repo: the per-NEFF uop table is generated from OPS at
    compile time, so appending here is equivalent to adding the constant
    to the file. sha pins are computed from the same lower() output that
    compile() checks against."""
    for op in _dve_ops.OPS:
        if op.name == name:
            return op
    spec = Spec(body=body, reference=ref)
    row = max(_dve_ops._SUB_OPCODE_FOR_NAME.values()) + 1
    assert row < 0x20, "custom-DVE row field overflow"
    _dve_ops._SUB_OPCODE_FOR_NAME[name] = row
    shas = {}
    for ver in ("v3", "v4"):
        uops = lower(spec, ver=ver)
        shas[ver] = DveOpSpec(
            name=name, opcode=row, uops=uops, rd1_en=_has_src1(spec)
        ).sha(ver)
    op = _dve_ops.DveOp(name, spec, subdim=False, uops_sha=shas)
    _dve_ops.OPS.append(op)
    _dve_ops.CUSTOM_DVE_SPECS[name] = spec
    return op


# A DVE instruction may read only ONE non-scalar input from PSUM
# (NCC_IBVF027), so the fused tree level-1 takes the raw second half-group
# from PSUM (Src0) and the already-relu'd first half (Src1, SBUF bf16):
RELU_PLUS = _register_op(
    "ANT_RELU_PLUS",
    relu(Src0 + C0) + Src1,
    lambda in0, in1, s0, s1, imm2: (
        np.maximum(in0 + s0, 0) + in1
    ).astype(np.float32),
)
RELU_MAXX = _register_op(
    "ANT_RELU_MAXX",
    maxx(relu(Src0 + C0), Src1),
    lambda in0, in1, s0, s1, imm2: np.maximum(
        np.maximum(in0 + s0, 0), in1
    ).astype(np.float32),
)


def _build(mode: str) -> bass.Bass:
    nc = bacc.Bacc(None, target_bir_lowering=False)
    f32 = mybir.dt.float32
    bf16 = mybir.dt.bfloat16
    ADD = mybir.AluOpType.add
    MAX = mybir.AluOpType.max
    RELU = mybir.ActivationFunctionType.Relu

    xt_d = nc.dram_tensor("xt", [IN_DIM, M_C], bf16, kind="ExternalInput")
    wt_d = nc.dram_tensor("wt", [IN_DIM, OUT_DIM], bf16, kind="ExternalInput")
    bsc_d = nc.dram_tensor("bsc", [128, N_CHUNK], f32, kind="ExternalInput")
    omax_d = nc.dram_tensor("omax", [OUT_DIM, G_C], f32, kind="ExternalOutput")
    osum_d = nc.dram_tensor("osum", [OUT_DIM, G_C], f32, kind="ExternalOutput")

    n_blk = (M_C + BLK - 1) // BLK
    GB = BLK // GS            # groups per full block

    with tile.TileContext(nc) as tc:
        with (
            tc.tile_pool(name="singles", bufs=1) as singles,
            tc.tile_pool(name="xin", bufs=3) as xin,
            tc.tile_pool(name="rsb", bufs=3) as rsb,
            tc.tile_pool(name="acc", bufs=1) as accp,
            tc.tile_pool(name="tree", bufs=4) as tpool,
            tc.tile_pool(name="psum", bufs=2, space="PSUM") as psum,
        ):
            wt_sb = singles.tile([IN_DIM, OUT_DIM], bf16)
            nc.sync.dma_start(out=wt_sb, in_=wt_d[:, :])
            bsc_sb = singles.tile([128, N_CHUNK], f32)
            nc.sync.dma_start(out=bsc_sb, in_=bsc_d[:, :])

            # persistent pooled accumulators [128, chunk, G_C]
            maxp_sb = accp.tile([128, N_CHUNK, G_C], f32)
            sump_sb = accp.tile([128, N_CHUNK, G_C], f32)

            # prime the ACT spline-table load while the first DMA is in flight
            warm_sb = singles.tile([128, 2], f32)
            nc.vector.memset(warm_sb, 0.0)
            nc.scalar.activation(
                out=warm_sb, in_=warm_sb, func=RELU, bias=0.0, scale=1.0,
            )

            blocks = [
                (i * BLK, min(BLK, M_C - i * BLK)) for i in range(n_blk)
            ]
            flush_after = {blocks[k][0] for k in (6, 12, 18, n_blk - 1)}
            flush_from = 0
            tree_ctr = 0

            for ib, (l0, lb) in enumerate(blocks):
                gb = lb // GS
                g0 = l0 // GS

                xt_sb = xin.tile([IN_DIM, BLK], bf16, tag="xt")
                nc.sync.dma_start(out=xt_sb[:, :lb], in_=xt_d[:, l0 : l0 + lb])

                n_wave = (lb + 511) // 512
                for c in range(N_CHUNK):
                    span = ib * N_CHUNK + c
                    enc_ps = psum.tile([128, BLK], f32, tag="enc")
                    for w in range(n_wave):
                        w0 = w * 512
                        lw = min(512, lb - w0)
                        nc.tensor.matmul(
                            enc_ps[:, w0 : w0 + lw],
                            wt_sb[:, c * 128 : (c + 1) * 128],
                            xt_sb[:, w0 : w0 + lw],
                            start=True, stop=True,
                        )
                    bias = bsc_sb[:, c : c + 1]
                    v3 = enc_ps[:, :lb].rearrange("p (g s) -> p g s", s=GS)

                    t1s = tpool.tile([128, GB, 8], bf16, tag="t1s")
                    t1m = tpool.tile([128, GB, 8], bf16, tag="t1m")
                    # sum tree engine rotates DVE/gpsimd; max tree is DVE-only
                    eng_s = (
                        nc.gpsimd if (tree_ctr % 10) < GPS_TENTHS else nc.vector
                    )
                    tree_ctr += 1
                    eng_m = nc.vector

                    if (span % 8) < C_EIGHTHS:
                        # C-span: ACT relus the first half-group; the custom
                        # DVE L1 fuses the second half's relu+bias with the
                        # pair combine (reads PSUM directly).
                        t0 = tpool.tile([128, GB, 8], bf16, tag="t0")
                        nc.scalar.activation(
                            out=t0[:, :gb, :], in_=v3[:, :, 0:8],
                            func=RELU, bias=bias, scale=1.0,
                        )
                        nc.vector._custom_dve(
                            RELU_PLUS,
                            out=t1s[:, :gb, :],
                            in0=v3[:, :, 8:16], in1=t0[:, :gb, :],
                            s0=bias,
                        )
                        nc.vector._custom_dve(
                            RELU_MAXX,
                            out=t1m[:, :gb, :],
                            in0=v3[:, :, 8:16], in1=t0[:, :gb, :],
                            s0=bias,
                        )
                    else:
                        # A-span: one full-width ACT relu -> r bf16; L1 on
                        # the tree engines.
                        r_sb = rsb.tile([128, BLK], bf16, tag="r")
                        nc.scalar.activation(
                            out=r_sb[:, :lb], in_=enc_ps[:, :lb],
                            func=RELU, bias=bias, scale=1.0,
                        )
                        r3 = r_sb[:, :lb].rearrange("p (g s) -> p g s", s=GS)
                        eng_s.tensor_tensor(
                            out=t1s[:, :gb, :],
                            in0=r3[:, :, 0:8], in1=r3[:, :, 8:16], op=ADD,
                        )
                        eng_m.tensor_tensor(
                            out=t1m[:, :gb, :],
                            in0=r3[:, :, 0:8], in1=r3[:, :, 8:16], op=MAX,
                        )

                    # tails: bf16 packed halves (DVE 2x mode); final level
                    # writes fp32 straight into the accumulators
                    t2s = tpool.tile([128, GB, 4], bf16, tag="t2s")
                    eng_s.tensor_tensor(
                        out=t2s[:, :gb, :],
                        in0=t1s[:, :gb, 0:4], in1=t1s[:, :gb, 4:8], op=ADD,
                    )
                    t3s = tpool.tile([128, GB, 2], bf16, tag="t3s")
                    eng_s.tensor_tensor(
                        out=t3s[:, :gb, :],
                        in0=t2s[:, :gb, 0:2], in1=t2s[:, :gb, 2:4], op=ADD,
                    )
                    eng_s.tensor_tensor(
                        out=sump_sb[:, c, g0 : g0 + gb],
                        in0=t3s[:, :gb, 0], in1=t3s[:, :gb, 1], op=ADD,
                    )

                    t2m = tpool.tile([128, GB, 4], bf16, tag="t2m")
                    eng_m.tensor_tensor(
                        out=t2m[:, :gb, :],
                        in0=t1m[:, :gb, 0:4], in1=t1m[:, :gb, 4:8], op=MAX,
                    )
                    t3m = tpool.tile([128, GB, 2], bf16, tag="t3m")
                    eng_m.tensor_tensor(
                        out=t3m[:, :gb, :],
                        in0=t2m[:, :gb, 0:2], in1=t2m[:, :gb, 2:4], op=MAX,
                    )
                    eng_m.tensor_tensor(
                        out=maxp_sb[:, c, g0 : g0 + gb],
                        in0=t3m[:, :gb, 0], in1=t3m[:, :gb, 1], op=MAX,
                    )

                # stream finished group ranges out so the kernel tail
                # overlaps with compute; no epilogue math needed
                if l0 in flush_after:
                    r0, r1 = flush_from, g0 + gb
                    flush_from = r1
                    for c in range(N_CHUNK):
                        nc.sync.dma_start(
                            out=omax_d[c * 128 : (c + 1) * 128, r0:r1],
                            in_=maxp_sb[:, c, r0:r1],
                        )
                        nc.sync.dma_start(
                            out=osum_d[c * 128 : (c + 1) * 128, r0:r1],
                            in_=sump_sb[:, c, r0:r1],
                        )

    nc.compile()
    return nc


def _get_nc(mode: str) -> bass.Bass:
    if mode not in _compiled:
        _compiled[mode] = _build(mode)
    return _compiled[mode]


def _host_prep(lane_encoding, W, b, mode: str):
    """Returns the per-core in_maps."""
    bf = ml_dtypes.bfloat16
    xT = np.ascontiguousarray(lane_encoding.T).astype(bf)   # [128, M] bf16
    wT = np.ascontiguousarray(W.T).astype(bf)               # [128, 512] bf16
    bsc = np.ascontiguousarray(b.reshape(N_CHUNK, 128).T.astype(np.float32))

    in_maps = []
    for c in range(N_CORES):
        sl = slice(c * M_C, (c + 1) * M_C)
        in_maps.append({
            "xt": np.ascontiguousarray(xT[:, sl]),
            "wt": wT, "bsc": bsc,
        })
    return in_maps


def _run(lane_encoding, W, b, mode: str = "fused", trace: bool = False):
    nc = _get_nc(mode)
    in_maps = _host_prep(lane_encoding, W, b, mode)
    try:
        res = run_bass_kernel_spmd(
            nc, in_maps, core_ids=list(range(N_CORES)), trace=trace
        )
    except Exception:
        # transient NRT_EXEC_UNIT_UNRECOVERABLE wedges have been observed;
        # a single retry usually succeeds
        res = run_bass_kernel_spmd(
            nc, in_maps, core_ids=list(range(N_CORES)), trace=trace
        )
    out = np.empty((N_OBS, 2 * OUT_DIM), dtype=np.float32)
    inv_gs = np.float32(1.0 / GS)
    for c in range(N_CORES):
        gsl = slice(c * G_C, (c + 1) * G_C)
        out[gsl, :OUT_DIM] = res.results[c]["omax"].T
        out[gsl, OUT_DIM:] = res.results[c]["osum"].T * inv_gs
    return out, res


MODE = "fused"


def kernel(obs_encoding, lane_encoding, same_obs_mask, W, b):
    out, _ = _run(
        np.asarray(lane_encoding, dtype=np.float32),
        np.asarray(W, dtype=np.float32),
        np.asarray(b, dtype=np.float32),
        MODE,
    )
    return out


# revision 8
# speedup vs baseline: 1.1329x; 1.1329x over previous
"""Trainium2 Bass kernel for AttentionalAggregation-style GNN pooling.

reference math:
    enc  = relu(lane_encoding @ W.T + b)            # [M=400000, 512]
    maxp = segment_max(enc, seg)                    # [N=25000, 512], 16 lanes/group
    avgp = segment_mean(enc, seg)                   # [N=25000, 512]
    out  = concat([maxp, avgp], axis=1)             # [N, 2*512]

Strategy (8 NeuronCores, data-parallel over lanes; each core owns whole groups):
  - Host pre-transposes x -> XT [128, M] bf16 so the contraction dim is the
    SBUF partition dim for both matmul operands. Single-pass bf16 matmul
    (~4e-3 rel err, gate is 2e-2) -> PE time is 1/3 of a compensated bf16x3.
  - ACT consumes PSUM: relu(u+b) -> r, a per-BLOCK [128, 4, 2048] bf16 tile
    (ACT is the only engine with a free fused relu+bias on the PSUM read).
  - Pooling is a radix-2 pairwise tree in bf16. Measured on HW: DVE
    tensor_tensor costs ~0.53 ns/output (2x mode) + ~150 ns/instruction,
    so the tree is fused across all 4 outdim chunks per block (4D APs,
    3 free dims) -> 4 instructions per tree per block instead of 16.
    TensorReduce has no fast modes (1.11 ns/input) and loses to the tree.
  - gpsimd (Pool) only implements tensor_tensor add/mult -- no max -- so
    the MAX tree is always DVE; a per-block knob routes SUM trees to
    gpsimd (~2.1 ns/elem, the slowest engine) to soak its capacity.
  - No on-device epilogues: max pool is exact (relu is monotone, bias
    inside), sum pool is divided by 16 on the host during reassembly.
  - Pooled outputs stay transposed ([512, G] fp32); host reassembles.
"""
import sys

sys.path.insert(0, "/opt/trn_rl_repo")

import numpy as np
import ml_dtypes

import concourse.bass as bass
import concourse.bacc as bacc
import concourse.tile as tile
from concourse import mybir
from concourse.bass_utils import run_bass_kernel_spmd

N_CORES = 8
IN_DIM = 128
OUT_DIM = 512
N_OBS = 25000
M_LANES = 400000
GS = 16                       # lanes per group
M_C = M_LANES // N_CORES      # 50000 lanes per core
G_C = N_OBS // N_CORES        # 3125 groups per core
N_CHUNK = OUT_DIM // 128      # 4 outdim chunks
BLK = 2048                    # lanes per DMA/compute block (4 psum banks)

# --- load-balance knobs (tuned against the ntff profile) -------------------
# SUM tree of block ib runs on gpsimd when (ib * GPS_NUM) % GPS_DEN < GPS_NUM
GPS_NUM = 10      # ~GPS_NUM/GPS_DEN of blocks' sum trees go to gpsimd
GPS_DEN = 25

_compiled = {}


def _build(mode: str) -> bass.Bass:
    nc = bacc.Bacc(None, target_bir_lowering=False)
    f32 = mybir.dt.float32
    bf16 = mybir.dt.bfloat16
    ADD = mybir.AluOpType.add
    MAX = mybir.AluOpType.max
    RELU = mybir.ActivationFunctionType.Relu

    xt_d = nc.dram_tensor("xt", [IN_DIM, M_C], bf16, kind="ExternalInput")
    wt_d = nc.dram_tensor("wt", [IN_DIM, OUT_DIM], bf16, kind="ExternalInput")
    bsc_d = nc.dram_tensor("bsc", [128, N_CHUNK], f32, kind="ExternalInput")
    omax_d = nc.dram_tensor("omax", [OUT_DIM, G_C], f32, kind="ExternalOutput")
    osum_d = nc.dram_tensor("osum", [OUT_DIM, G_C], f32, kind="ExternalOutput")

    n_blk = (M_C + BLK - 1) // BLK
    GB = BLK // GS            # groups per full block

    with tile.TileContext(nc) as tc:
        with (
            tc.tile_pool(name="singles", bufs=1) as singles,
            tc.tile_pool(name="xin", bufs=3) as xin,
            tc.tile_pool(name="rsb", bufs=3) as rsb,
            tc.tile_pool(name="tree", bufs=3) as tpool,
            tc.tile_pool(name="psum", bufs=2, space="PSUM") as psum,
        ):
            wt_sb = singles.tile([IN_DIM, OUT_DIM], bf16)
            nc.sync.dma_start(out=wt_sb, in_=wt_d[:, :])
            bsc_sb = singles.tile([128, N_CHUNK], f32)
            nc.sync.dma_start(out=bsc_sb, in_=bsc_d[:, :])

            # prime the ACT spline-table load while the first DMA is in flight
            warm_sb = singles.tile([128, 2], f32)
            nc.vector.memset(warm_sb, 0.0)
            nc.scalar.activation(
                out=warm_sb, in_=warm_sb, func=RELU, bias=0.0, scale=1.0,
            )

            blocks = [
                (i * BLK, min(BLK, M_C - i * BLK)) for i in range(n_blk)
            ]

            for ib, (l0, lb) in enumerate(blocks):
                gb = lb // GS
                g0 = l0 // GS

                xt_sb = xin.tile([IN_DIM, BLK], bf16, tag="xt")
                nc.sync.dma_start(out=xt_sb[:, :lb], in_=xt_d[:, l0 : l0 + lb])

                # per-block relu'd activations, all 4 chunks: [128, 4, BLK]
                r_sb = rsb.tile([128, N_CHUNK, BLK], bf16, tag="r")

                n_wave = (lb + 511) // 512
                for c in range(N_CHUNK):
                    enc_ps = psum.tile([128, BLK], f32, tag="enc")
                    for w in range(n_wave):
                        w0 = w * 512
                        lw = min(512, lb - w0)
                        nc.tensor.matmul(
                            enc_ps[:, w0 : w0 + lw],
                            wt_sb[:, c * 128 : (c + 1) * 128],
                            xt_sb[:, w0 : w0 + lw],
                            start=True, stop=True,
                        )
                    nc.scalar.activation(
                        out=r_sb[:, c, :lb], in_=enc_ps[:, :lb],
                        func=RELU, bias=bsc_sb[:, c : c + 1], scale=1.0,
                    )

                # block-fused pooling trees over [128, 4, gb, 16]
                r4 = r_sb[:, :, :lb].rearrange("p c (g s) -> p c g s", s=GS)
                eng_s = (
                    nc.gpsimd if (ib * GPS_NUM) % GPS_DEN < GPS_NUM
                    else nc.vector
                )

                t1s = tpool.tile([128, N_CHUNK, GB, 8], bf16, tag="t1s")
                t2s = tpool.tile([128, N_CHUNK, GB, 4], bf16, tag="t2s")
                t3s = tpool.tile([128, N_CHUNK, GB, 2], bf16, tag="t3s")
                eng_s.tensor_tensor(
                    out=t1s[:, :, :gb, :],
                    in0=r4[:, :, :, 0:8], in1=r4[:, :, :, 8:16], op=ADD,
                )
                eng_s.tensor_tensor(
                    out=t2s[:, :, :gb, :],
                    in0=t1s[:, :, :gb, 0:4], in1=t1s[:, :, :gb, 4:8], op=ADD,
                )
                eng_s.tensor_tensor(
                    out=t3s[:, :, :gb, :],
                    in0=t2s[:, :, :gb, 0:2], in1=t2s[:, :, :gb, 2:4], op=ADD,
                )
                psum_o = tpool.tile([128, N_CHUNK, GB], f32, tag="psum_o")
                eng_s.tensor_tensor(
                    out=psum_o[:, :, :gb],
                    in0=t3s[:, :, :gb, 0], in1=t3s[:, :, :gb, 1], op=ADD,
                )

                t1m = tpool.tile([128, N_CHUNK, GB, 8], bf16, tag="t1m")
                t2m = tpool.tile([128, N_CHUNK, GB, 4], bf16, tag="t2m")
                t3m = tpool.tile([128, N_CHUNK, GB, 2], bf16, tag="t3m")
                nc.vector.tensor_tensor(
                    out=t1m[:, :, :gb, :],
                    in0=r4[:, :, :, 0:8], in1=r4[:, :, :, 8:16], op=MAX,
                )
                nc.vector.tensor_tensor(
                    out=t2m[:, :, :gb, :],
                    in0=t1m[:, :, :gb, 0:4], in1=t1m[:, :, :gb, 4:8], op=MAX,
                )
                nc.vector.tensor_tensor(
                    out=t3m[:, :, :gb, :],
                    in0=t2m[:, :, :gb, 0:2], in1=t2m[:, :, :gb, 2:4], op=MAX,
                )
                pmax_o = tpool.tile([128, N_CHUNK, GB], f32, tag="pmax_o")
                nc.vector.tensor_tensor(
                    out=pmax_o[:, :, :gb],
                    in0=t3m[:, :, :gb, 0], in1=t3m[:, :, :gb, 1], op=MAX,
                )

                # stream this block's pooled groups straight out (no
                # accumulators -> 100KB/partition of SBUF saved)
                for c in range(N_CHUNK):
                    nc.sync.dma_start(
                        out=omax_d[c * 128 : (c + 1) * 128, g0 : g0 + gb],
                        in_=pmax_o[:, c, :gb],
                    )
                    nc.sync.dma_start(
                        out=osum_d[c * 128 : (c + 1) * 128, g0 : g0 + gb],
                        in_=psum_o[:, c, :gb],
                    )

    nc.compile()
    return nc


def _get_nc(mode: str) -> bass.Bass:
    if mode not in _compiled:
        _compiled[mode] = _build(mode)
    return _compiled[mode]


def _host_prep(lane_encoding, W, b, mode: str):
    """Returns the per-core in_maps."""
    bf = ml_dtypes.bfloat16
    xT = np.ascontiguousarray(lane_encoding.T).astype(bf)   # [128, M] bf16
    wT = np.ascontiguousarray(W.T).astype(bf)               # [128, 512] bf16
    bsc = np.ascontiguousarray(b.reshape(N_CHUNK, 128).T.astype(np.float32))

    in_maps = []
    for c in range(N_CORES):
        sl = slice(c * M_C, (c + 1) * M_C)
        in_maps.append({
            "xt": np.ascontiguousarray(xT[:, sl]),
            "wt": wT, "bsc": bsc,
        })
    return in_maps


def _run(lane_encoding, W, b, mode: str = "fused", trace: bool = False):
    nc = _get_nc(mode)
    in_maps = _host_prep(lane_encoding, W, b, mode)
    try:
        res = run_bass_kernel_spmd(
            nc, in_maps, core_ids=list(range(N_CORES)), trace=trace
        )
    except Exception:
        # transient NRT_EXEC_UNIT_UNRECOVERABLE wedges have been observed;
        # a single retry usually succeeds
        res = run_bass_kernel_spmd(
            nc, in_maps, core_ids=list(range(N_CORES)), trace=trace
        )
    out = np.empty((N_OBS, 2 * OUT_DIM), dtype=np.float32)
    inv_gs = np.float32(1.0 / GS)
    for c in range(N_CORES):
        gsl = slice(c * G_C, (c + 1) * G_C)
        out[gsl, :OUT_DIM] = res.results[c]["omax"].T
        out[gsl, OUT_DIM:] = res.results[c]["osum"].T * inv_gs
    return out, res


MODE = "fused"


def kernel(obs_encoding, lane_encoding, same_obs_mask, W, b):
    out, _ = _run(
        np.asarray(lane_encoding, dtype=np.float32),
        np.asarray(W, dtype=np.float32),
        np.asarray(b, dtype=np.float32),
        MODE,
    )
    return out


# revision 9
# speedup vs baseline: 1.1570x; 1.0213x over previous
"""Trainium2 Bass kernel for AttentionalAggregation-style GNN pooling.

reference math:
    enc  = relu(lane_encoding @ W.T + b)            # [M=400000, 512]
    maxp = segment_max(enc, seg)                    # [N=25000, 512], 16 lanes/group
    avgp = segment_mean(enc, seg)                   # [N=25000, 512]
    out  = concat([maxp, avgp], axis=1)             # [N, 2*512]

Strategy (8 NeuronCores, data-parallel over lanes; each core owns whole groups):
  - Host pre-transposes x -> XT [128, M] bf16 so the contraction dim is the
    SBUF partition dim for both matmul operands. Single-pass bf16 matmul
    (~4e-3 rel err, gate is 2e-2) -> PE time is 1/3 of a compensated bf16x3.
  - ACT consumes PSUM: relu(u+b) -> r, a per-BLOCK [128, 4, 2048] bf16 tile
    (ACT is the only engine with a free fused relu+bias on the PSUM read).
  - Pooling is a radix-2 pairwise tree in bf16. Measured on HW: DVE
    tensor_tensor costs ~0.53 ns/output (2x mode) + ~150 ns/instruction,
    so the tree is fused across all 4 outdim chunks per block (4D APs,
    3 free dims) -> 4 instructions per tree per block instead of 16.
    TensorReduce has no fast modes (1.11 ns/input) and loses to the tree.
  - gpsimd (Pool) only implements tensor_tensor add/mult -- no max -- so
    the MAX tree is always DVE; a per-block knob routes SUM trees to
    gpsimd (~2.1 ns/elem, the slowest engine) to soak its capacity.
  - No on-device epilogues: max pool is exact (relu is monotone, bias
    inside), sum pool is divided by 16 on the host during reassembly.
  - Pooled outputs stay transposed ([512, G] fp32); host reassembles.
"""
import sys

sys.path.insert(0, "/opt/trn_rl_repo")

import numpy as np
import ml_dtypes

import concourse.bass as bass
import concourse.bacc as bacc
import concourse.tile as tile
from concourse import mybir
from concourse.bass_utils import run_bass_kernel_spmd

N_CORES = 8
IN_DIM = 128
OUT_DIM = 512
N_OBS = 25000
M_LANES = 400000
GS = 16                       # lanes per group
M_C = M_LANES // N_CORES      # 50000 lanes per core
G_C = N_OBS // N_CORES        # 3125 groups per core
N_CHUNK = OUT_DIM // 128      # 4 outdim chunks
BLK = 2048                    # lanes per DMA/compute block (4 psum banks)

# --- load-balance knobs (tuned against the ntff profile) -------------------
# SUM tree of block ib runs on gpsimd when (ib * GPS_NUM) % GPS_DEN < GPS_NUM
GPS_NUM = 10      # ~GPS_NUM/GPS_DEN of blocks' sum trees go to gpsimd
GPS_DEN = 25

_compiled = {}


def _build(mode: str) -> bass.Bass:
    nc = bacc.Bacc(None, target_bir_lowering=False)
    f32 = mybir.dt.float32
    bf16 = mybir.dt.bfloat16
    ADD = mybir.AluOpType.add
    MAX = mybir.AluOpType.max
    RELU = mybir.ActivationFunctionType.Relu

    xt_d = nc.dram_tensor("xt", [IN_DIM, M_C], bf16, kind="ExternalInput")
    wt_d = nc.dram_tensor("wt", [IN_DIM, OUT_DIM], bf16, kind="ExternalInput")
    bsc_d = nc.dram_tensor("bsc", [128, N_CHUNK], f32, kind="ExternalInput")
    omax_d = nc.dram_tensor("omax", [OUT_DIM, G_C], f32, kind="ExternalOutput")
    osum_d = nc.dram_tensor("osum", [OUT_DIM, G_C], f32, kind="ExternalOutput")

    n_blk = (M_C + BLK - 1) // BLK
    GB = BLK // GS            # groups per full block

    with tile.TileContext(nc) as tc:
        with (
            tc.tile_pool(name="singles", bufs=1) as singles,
            tc.tile_pool(name="xin", bufs=3) as xin,
            tc.tile_pool(name="rsb", bufs=3) as rsb,
            tc.tile_pool(name="tree", bufs=3) as tpool,
            tc.tile_pool(name="psum", bufs=2, space="PSUM") as psum,
        ):
            wt_sb = singles.tile([IN_DIM, OUT_DIM], bf16)
            nc.sync.dma_start(out=wt_sb, in_=wt_d[:, :])
            bsc_sb = singles.tile([128, N_CHUNK], f32)
            nc.sync.dma_start(out=bsc_sb, in_=bsc_d[:, :])

            # prime the ACT spline-table load while the first DMA is in flight
            warm_sb = singles.tile([128, 2], f32)
            nc.vector.memset(warm_sb, 0.0)
            nc.scalar.activation(
                out=warm_sb, in_=warm_sb, func=RELU, bias=0.0, scale=1.0,
            )

            blocks = [
                (i * BLK, min(BLK, M_C - i * BLK)) for i in range(n_blk)
            ]

            for ib, (l0, lb) in enumerate(blocks):
                gb = lb // GS
                g0 = l0 // GS

                xt_sb = xin.tile([IN_DIM, BLK], bf16, tag="xt")
                nc.sync.dma_start(out=xt_sb[:, :lb], in_=xt_d[:, l0 : l0 + lb])

                # per-block relu'd activations, all 4 chunks: [128, 4, BLK]
                r_sb = rsb.tile([128, N_CHUNK, BLK], bf16, tag="r")

                n_wave = (lb + 511) // 512
                for c in range(N_CHUNK):
                    enc_ps = psum.tile([128, BLK], f32, tag="enc")
                    for w in range(n_wave):
                        w0 = w * 512
                        lw = min(512, lb - w0)
                        nc.tensor.matmul(
                            enc_ps[:, w0 : w0 + lw],
                            wt_sb[:, c * 128 : (c + 1) * 128],
                            xt_sb[:, w0 : w0 + lw],
                            start=True, stop=True,
                        )
                    nc.scalar.activation(
                        out=r_sb[:, c, :lb], in_=enc_ps[:, :lb],
                        func=RELU, bias=bsc_sb[:, c : c + 1], scale=1.0,
                    )

                # Block-fused pooling trees. 4D APs run ~4x slower on DVE,
                # so flatten (chunk, group) into ONE dim — exact for full
                # blocks (gb == GB makes the strides uniform); the last
                # partial block falls back to per-chunk 3D trees.
                eng_s = (
                    nc.gpsimd if (ib * GPS_NUM) % GPS_DEN < GPS_NUM
                    else nc.vector
                )
                psum_o = tpool.tile([128, N_CHUNK, GB], f32, tag="psum_o")
                pmax_o = tpool.tile([128, N_CHUNK, GB], f32, tag="pmax_o")
                t1s = tpool.tile([128, N_CHUNK * GB, 8], bf16, tag="t1s")
                t2s = tpool.tile([128, N_CHUNK * GB, 4], bf16, tag="t2s")
                t3s = tpool.tile([128, N_CHUNK * GB, 2], bf16, tag="t3s")
                t1m = tpool.tile([128, N_CHUNK * GB, 8], bf16, tag="t1m")
                t2m = tpool.tile([128, N_CHUNK * GB, 4], bf16, tag="t2m")
                t3m = tpool.tile([128, N_CHUNK * GB, 2], bf16, tag="t3m")

                if gb == GB:
                    views = [(
                        r_sb.rearrange("p c (g s) -> p (c g) s", s=GS),
                        t1s, t2s, t3s,
                        psum_o.rearrange("p c g -> p (c g)"),
                        t1m, t2m, t3m,
                        pmax_o.rearrange("p c g -> p (c g)"),
                    )]
                else:
                    views = []
                    for c in range(N_CHUNK):
                        cs = slice(c * GB, c * GB + gb)
                        views.append((
                            r_sb[:, c, :lb].rearrange(
                                "p (g s) -> p g s", s=GS),
                            t1s[:, cs, :], t2s[:, cs, :], t3s[:, cs, :],
                            psum_o[:, c, :gb],
                            t1m[:, cs, :], t2m[:, cs, :], t3m[:, cs, :],
                            pmax_o[:, c, :gb],
                        ))

                for rv, u1, u2, u3, so, v1, v2, v3, mo in views:
                    eng_s.tensor_tensor(
                        out=u1, in0=rv[:, :, 0:8], in1=rv[:, :, 8:16], op=ADD)
                    eng_s.tensor_tensor(
                        out=u2, in0=u1[:, :, 0:4], in1=u1[:, :, 4:8], op=ADD)
                    eng_s.tensor_tensor(
                        out=u3, in0=u2[:, :, 0:2], in1=u2[:, :, 2:4], op=ADD)
                    eng_s.tensor_tensor(
                        out=so, in0=u3[:, :, 0], in1=u3[:, :, 1], op=ADD)
                    nc.vector.tensor_tensor(
                        out=v1, in0=rv[:, :, 0:8], in1=rv[:, :, 8:16], op=MAX)
                    nc.vector.tensor_tensor(
                        out=v2, in0=v1[:, :, 0:4], in1=v1[:, :, 4:8], op=MAX)
                    nc.vector.tensor_tensor(
                        out=v3, in0=v2[:, :, 0:2], in1=v2[:, :, 2:4], op=MAX)
                    nc.vector.tensor_tensor(
                        out=mo, in0=v3[:, :, 0], in1=v3[:, :, 1], op=MAX)

                # stream this block's pooled groups straight out; one DMA
                # per output covers all 4 chunks (regular row stride)
                omax_v = omax_d.rearrange("(c p) g -> p c g", p=128)
                osum_v = osum_d.rearrange("(c p) g -> p c g", p=128)
                nc.sync.dma_start(
                    out=omax_v[:, :, g0 : g0 + gb], in_=pmax_o[:, :, :gb])
                nc.sync.dma_start(
                    out=osum_v[:, :, g0 : g0 + gb], in_=psum_o[:, :, :gb])

    nc.compile()
    return nc


def _get_nc(mode: str) -> bass.Bass:
    if mode not in _compiled:
        _compiled[mode] = _build(mode)
    return _compiled[mode]


def _host_prep(lane_encoding, W, b, mode: str):
    """Returns the per-core in_maps."""
    bf = ml_dtypes.bfloat16
    xT = np.ascontiguousarray(lane_encoding.T).astype(bf)   # [128, M] bf16
    wT = np.ascontiguousarray(W.T).astype(bf)               # [128, 512] bf16
    bsc = np.ascontiguousarray(b.reshape(N_CHUNK, 128).T.astype(np.float32))

    in_maps = []
    for c in range(N_CORES):
        sl = slice(c * M_C, (c + 1) * M_C)
        in_maps.append({
            "xt": np.ascontiguousarray(xT[:, sl]),
            "wt": wT, "bsc": bsc,
        })
    return in_maps


def _run(lane_encoding, W, b, mode: str = "fused", trace: bool = False):
    nc = _get_nc(mode)
    in_maps = _host_prep(lane_encoding, W, b, mode)
    try:
        res = run_bass_kernel_spmd(
            nc, in_maps, core_ids=list(range(N_CORES)), trace=trace
        )
    except Exception:
        # transient NRT_EXEC_UNIT_UNRECOVERABLE wedges have been observed;
        # a single retry usually succeeds
        res = run_bass_kernel_spmd(
            nc, in_maps, core_ids=list(range(N_CORES)), trace=trace
        )
    out = np.empty((N_OBS, 2 * OUT_DIM), dtype=np.float32)
    inv_gs = np.float32(1.0 / GS)
    for c in range(N_CORES):
        gsl = slice(c * G_C, (c + 1) * G_C)
        out[gsl, :OUT_DIM] = res.results[c]["omax"].T
        out[gsl, OUT_DIM:] = res.results[c]["osum"].T * inv_gs
    return out, res


MODE = "fused"


def kernel(obs_encoding, lane_encoding, same_obs_mask, W, b):
    out, _ = _run(
        np.asarray(lane_encoding, dtype=np.float32),
        np.asarray(W, dtype=np.float32),
        np.asarray(b, dtype=np.float32),
        MODE,
    )
    return out


# revision 10
# speedup vs baseline: 1.1588x; 1.0015x over previous
"""Trainium2 Bass kernel for AttentionalAggregation-style GNN pooling.

reference math:
    enc  = relu(lane_encoding @ W.T + b)            # [M=400000, 512]
    maxp = segment_max(enc, seg)                    # [N=25000, 512], 16 lanes/group
    avgp = segment_mean(enc, seg)                   # [N=25000, 512]
    out  = concat([maxp, avgp], axis=1)             # [N, 2*512]

Strategy (8 NeuronCores, data-parallel over lanes; each core owns whole groups):
  - Host pre-transposes x -> XT [128, M] bf16 so the contraction dim is the
    SBUF partition dim for both matmul operands. Single-pass bf16 matmul
    (~4e-3 rel err, gate is 2e-2) -> PE time is 1/3 of a compensated bf16x3.
  - ACT consumes PSUM: relu(u+b) -> r, a per-BLOCK [128, 4, 2048] bf16 tile
    (ACT is the only engine with a free fused relu+bias on the PSUM read).
  - Pooling is a radix-2 pairwise tree in bf16. Measured on HW: DVE
    tensor_tensor costs ~0.53 ns/output (2x mode) + ~150 ns/instruction,
    so the tree is fused across all 4 outdim chunks per block (4D APs,
    3 free dims) -> 4 instructions per tree per block instead of 16.
    TensorReduce has no fast modes (1.11 ns/input) and loses to the tree.
  - gpsimd (Pool) only implements tensor_tensor add/mult -- no max -- so
    the MAX tree is always DVE; a per-block knob routes SUM trees to
    gpsimd (~2.1 ns/elem, the slowest engine) to soak its capacity.
  - No on-device epilogues: max pool is exact (relu is monotone, bias
    inside), sum pool is divided by 16 on the host during reassembly.
  - Pooled outputs stay transposed ([512, G] fp32); host reassembles.
"""
import sys

sys.path.insert(0, "/opt/trn_rl_repo")

import numpy as np
import ml_dtypes

import concourse.bass as bass
import concourse.bacc as bacc
import concourse.tile as tile
from concourse import mybir
from concourse.bass_utils import run_bass_kernel_spmd

N_CORES = 8
IN_DIM = 128
OUT_DIM = 512
N_OBS = 25000
M_LANES = 400000
GS = 16                       # lanes per group
M_C = M_LANES // N_CORES      # 50000 lanes per core
G_C = N_OBS // N_CORES        # 3125 groups per core
N_CHUNK = OUT_DIM // 128      # 4 outdim chunks
BLK = 2048                    # lanes per DMA/compute block (4 psum banks)

# --- load-balance knobs (tuned against the ntff profile) -------------------
# SUM tree of block ib runs on gpsimd when (ib * GPS_NUM) % GPS_DEN < GPS_NUM
GPS_NUM = 10      # ~GPS_NUM/GPS_DEN of blocks' sum trees go to gpsimd
GPS_DEN = 25

_compiled = {}


def _build(mode: str) -> bass.Bass:
    nc = bacc.Bacc(None, target_bir_lowering=False)
    f32 = mybir.dt.float32
    bf16 = mybir.dt.bfloat16
    ADD = mybir.AluOpType.add
    MAX = mybir.AluOpType.max
    RELU = mybir.ActivationFunctionType.Relu

    xt_d = nc.dram_tensor("xt", [IN_DIM, M_C], bf16, kind="ExternalInput")
    wt_d = nc.dram_tensor("wt", [IN_DIM, OUT_DIM], bf16, kind="ExternalInput")
    bsc_d = nc.dram_tensor("bsc", [128, N_CHUNK], f32, kind="ExternalInput")
    omax_d = nc.dram_tensor("omax", [OUT_DIM, G_C], f32, kind="ExternalOutput")
    osum_d = nc.dram_tensor("osum", [OUT_DIM, G_C], f32, kind="ExternalOutput")

    n_blk = (M_C + BLK - 1) // BLK
    GB = BLK // GS            # groups per full block

    with tile.TileContext(nc) as tc:
        with (
            tc.tile_pool(name="singles", bufs=1) as singles,
            tc.tile_pool(name="xin", bufs=3) as xin,
            tc.tile_pool(name="rsb", bufs=3) as rsb,
            tc.tile_pool(name="tree", bufs=3) as tpool,
            tc.tile_pool(name="psum", bufs=2, space="PSUM") as psum,
        ):
            wt_sb = singles.tile([IN_DIM, OUT_DIM], bf16)
            nc.sync.dma_start(out=wt_sb, in_=wt_d[:, :])
            bsc_sb = singles.tile([128, N_CHUNK], f32)
            nc.sync.dma_start(out=bsc_sb, in_=bsc_d[:, :])

            # prime the ACT spline-table load while the first DMA is in flight
            warm_sb = singles.tile([128, 2], f32)
            nc.vector.memset(warm_sb, 0.0)
            nc.scalar.activation(
                out=warm_sb, in_=warm_sb, func=RELU, bias=0.0, scale=1.0,
            )

            blocks = [
                (i * BLK, min(BLK, M_C - i * BLK)) for i in range(n_blk)
            ]

            for ib, (l0, lb) in enumerate(blocks):
                gb = lb // GS
                g0 = l0 // GS

                xt_sb = xin.tile([IN_DIM, BLK], bf16, tag="xt")
                nc.sync.dma_start(out=xt_sb[:, :lb], in_=xt_d[:, l0 : l0 + lb])

                # per-block relu'd activations, all 4 chunks: [128, 4, BLK]
                r_sb = rsb.tile([128, N_CHUNK, BLK], bf16, tag="r")

                n_wave = (lb + 511) // 512
                for c in range(N_CHUNK):
                    enc_ps = psum.tile([128, BLK], f32, tag="enc")
                    for w in range(n_wave):
                        w0 = w * 512
                        lw = min(512, lb - w0)
                        nc.tensor.matmul(
                            enc_ps[:, w0 : w0 + lw],
                            wt_sb[:, c * 128 : (c + 1) * 128],
                            xt_sb[:, w0 : w0 + lw],
                            start=True, stop=True,
                        )
                    nc.scalar.activation(
                        out=r_sb[:, c, :lb], in_=enc_ps[:, :lb],
                        func=RELU, bias=bsc_sb[:, c : c + 1], scale=1.0,
                    )

                # Block-fused pooling trees. 4D APs run ~4x slower on DVE,
                # so flatten (chunk, group) into ONE dim — exact for full
                # blocks (gb == GB makes the strides uniform); the last
                # partial block falls back to per-chunk 3D trees.
                eng_s = (
                    nc.gpsimd if (ib * GPS_NUM) % GPS_DEN < GPS_NUM
                    else nc.vector
                )
                psum_o = tpool.tile([128, N_CHUNK, GB], f32, tag="psum_o")
                pmax_o = tpool.tile([128, N_CHUNK, GB], f32, tag="pmax_o")
                t1s = tpool.tile([128, N_CHUNK * GB, 8], bf16, tag="t1s")
                t2s = tpool.tile([128, N_CHUNK * GB, 4], bf16, tag="t2s")
                t3s = tpool.tile([128, N_CHUNK * GB, 2], bf16, tag="t3s")
                t1m = tpool.tile([128, N_CHUNK * GB, 8], bf16, tag="t1m")
                t2m = tpool.tile([128, N_CHUNK * GB, 4], bf16, tag="t2m")
                t3m = tpool.tile([128, N_CHUNK * GB, 2], bf16, tag="t3m")

                if gb == GB:
                    views = [(
                        r_sb.rearrange("p c (g s) -> p (c g) s", s=GS),
                        t1s, t2s, t3s,
                        psum_o.rearrange("p c g -> p (c g)"),
                        t1m, t2m, t3m,
                        pmax_o.rearrange("p c g -> p (c g)"),
                    )]
                else:
                    views = []
                    for c in range(N_CHUNK):
                        cs = slice(c * GB, c * GB + gb)
                        views.append((
                            r_sb[:, c, :lb].rearrange(
                                "p (g s) -> p g s", s=GS),
                            t1s[:, cs, :], t2s[:, cs, :], t3s[:, cs, :],
                            psum_o[:, c, :gb],
                            t1m[:, cs, :], t2m[:, cs, :], t3m[:, cs, :],
                            pmax_o[:, c, :gb],
                        ))

                # issue order interleaves the two independent chains so
                # back-to-back DVE instructions never depend on each other
                # (a chained op stalls ~0.5us on its producer's write-ack)
                for rv, u1, u2, u3, so, v1, v2, v3, mo in views:
                    nc.vector.tensor_tensor(
                        out=v1, in0=rv[:, :, 0:8], in1=rv[:, :, 8:16], op=MAX)
                    eng_s.tensor_tensor(
                        out=u1, in0=rv[:, :, 0:8], in1=rv[:, :, 8:16], op=ADD)
                    nc.vector.tensor_tensor(
                        out=v2, in0=v1[:, :, 0:4], in1=v1[:, :, 4:8], op=MAX)
                    eng_s.tensor_tensor(
                        out=u2, in0=u1[:, :, 0:4], in1=u1[:, :, 4:8], op=ADD)
                    nc.vector.tensor_tensor(
                        out=v3, in0=v2[:, :, 0:2], in1=v2[:, :, 2:4], op=MAX)
                    eng_s.tensor_tensor(
                        out=u3, in0=u2[:, :, 0:2], in1=u2[:, :, 2:4], op=ADD)
                    nc.vector.tensor_tensor(
                        out=mo, in0=v3[:, :, 0], in1=v3[:, :, 1], op=MAX)
                    eng_s.tensor_tensor(
                        out=so, in0=u3[:, :, 0], in1=u3[:, :, 1], op=ADD)

                # stream this block's pooled groups straight out; one DMA
                # per output covers all 4 chunks (regular row stride)
                omax_v = omax_d.rearrange("(c p) g -> p c g", p=128)
                osum_v = osum_d.rearrange("(c p) g -> p c g", p=128)
                nc.sync.dma_start(
                    out=omax_v[:, :, g0 : g0 + gb], in_=pmax_o[:, :, :gb])
                nc.sync.dma_start(
                    out=osum_v[:, :, g0 : g0 + gb], in_=psum_o[:, :, :gb])

    nc.compile()
    return nc


def _get_nc(mode: str) -> bass.Bass:
    if mode not in _compiled:
        _compiled[mode] = _build(mode)
    return _compiled[mode]


def _host_prep(lane_encoding, W, b, mode: str):
    """Returns the per-core in_maps."""
    bf = ml_dtypes.bfloat16
    xT = np.ascontiguousarray(lane_encoding.T).astype(bf)   # [128, M] bf16
    wT = np.ascontiguousarray(W.T).astype(bf)               # [128, 512] bf16
    bsc = np.ascontiguousarray(b.reshape(N_CHUNK, 128).T.astype(np.float32))

    in_maps = []
    for c in range(N_CORES):
        sl = slice(c * M_C, (c + 1) * M_C)
        in_maps.append({
            "xt": np.ascontiguousarray(xT[:, sl]),
            "wt": wT, "bsc": bsc,
        })
    return in_maps


def _run(lane_encoding, W, b, mode: str = "fused", trace: bool = False):
    nc = _get_nc(mode)
    in_maps = _host_prep(lane_encoding, W, b, mode)
    try:
        res = run_bass_kernel_spmd(
            nc, in_maps, core_ids=list(range(N_CORES)), trace=trace
        )
    except Exception:
        # transient NRT_EXEC_UNIT_UNRECOVERABLE wedges have been observed;
        # a single retry usually succeeds
        res = run_bass_kernel_spmd(
            nc, in_maps, core_ids=list(range(N_CORES)), trace=trace
        )
    out = np.empty((N_OBS, 2 * OUT_DIM), dtype=np.float32)
    inv_gs = np.float32(1.0 / GS)
    for c in range(N_CORES):
        gsl = slice(c * G_C, (c + 1) * G_C)
        out[gsl, :OUT_DIM] = res.results[c]["omax"].T
        out[gsl, OUT_DIM:] = res.results[c]["osum"].T * inv_gs
    return out, res


MODE = "fused"


def kernel(obs_encoding, lane_encoding, same_obs_mask, W, b):
    out, _ = _run(
        np.asarray(lane_encoding, dtype=np.float32),
        np.asarray(W, dtype=np.float32),
        np.asarray(b, dtype=np.float32),
        MODE,
    )
    return out


# revision 11
# speedup vs baseline: 1.5881x; 1.3705x over previous
"""Trainium2 Bass kernel for AttentionalAggregation-style GNN pooling.

reference math:
    enc  = relu(lane_encoding @ W.T + b)            # [M=400000, 512]
    maxp = segment_max(enc, seg)                    # [N=25000, 512], 16 lanes/group
    avgp = segment_mean(enc, seg)                   # [N=25000, 512]
    out  = concat([maxp, avgp], axis=1)             # [N, 2*512]

Strategy (8 NeuronCores, data-parallel over lanes; each core owns whole groups):
  - Host pre-transposes x -> XT [128, M] bf16 so the contraction dim is the
    SBUF partition dim for both matmul operands. Single-pass bf16 matmul
    (~4e-3 rel err, gate is 2e-2) -> PE time is 1/3 of a compensated bf16x3.
  - ACT consumes PSUM: relu(u+b) -> r, a per-BLOCK [128, 4, 2048] bf16 tile
    (ACT is the only engine with a free fused relu+bias on the PSUM read).
  - Pooling is a radix-2 pairwise tree in bf16. Measured on HW: DVE
    tensor_tensor costs ~0.53 ns/output (2x mode) + ~150 ns/instruction,
    so the tree is fused across all 4 outdim chunks per block (4D APs,
    3 free dims) -> 4 instructions per tree per block instead of 16.
    TensorReduce has no fast modes (1.11 ns/input) and loses to the tree.
  - gpsimd (Pool) only implements tensor_tensor add/mult -- no max -- so
    the MAX tree is always DVE; a per-block knob routes SUM trees to
    gpsimd (~2.1 ns/elem, the slowest engine) to soak its capacity.
  - No on-device epilogues: max pool is exact (relu is monotone, bias
    inside), sum pool is divided by 16 on the host during reassembly.
  - Pooled outputs stay transposed ([512, G] fp32); host reassembles.
"""
import sys

sys.path.insert(0, "/opt/trn_rl_repo")

import numpy as np
import ml_dtypes

import concourse.bass as bass
import concourse.bacc as bacc
import concourse.tile as tile
from concourse import mybir
from concourse.bass_utils import run_bass_kernel_spmd

N_CORES = 8
IN_DIM = 128
OUT_DIM = 512
N_OBS = 25000
M_LANES = 400000
GS = 16                       # lanes per group
M_C = M_LANES // N_CORES      # 50000 lanes per core
G_C = N_OBS // N_CORES        # 3125 groups per core
N_CHUNK = OUT_DIM // 128      # 4 outdim chunks
BLK = 2048                    # lanes per DMA/compute block (4 psum banks)

# --- load-balance knobs (tuned against the ntff profile) -------------------
# SUM tree of block ib runs on gpsimd when (ib * GPS_NUM) % GPS_DEN < GPS_NUM
GPS_NUM = 0      # ~GPS_NUM/GPS_DEN of blocks' sum trees go to gpsimd
GPS_DEN = 25

_compiled = {}


def _build(mode: str) -> bass.Bass:
    nc = bacc.Bacc(None, target_bir_lowering=False)
    f32 = mybir.dt.float32
    bf16 = mybir.dt.bfloat16
    ADD = mybir.AluOpType.add
    MAX = mybir.AluOpType.max
    RELU = mybir.ActivationFunctionType.Relu

    xt_d = nc.dram_tensor("xt", [IN_DIM, M_C], bf16, kind="ExternalInput")
    wt_d = nc.dram_tensor("wt", [IN_DIM, OUT_DIM], bf16, kind="ExternalInput")
    bsc_d = nc.dram_tensor("bsc", [128, N_CHUNK], f32, kind="ExternalInput")
    omax_d = nc.dram_tensor("omax", [OUT_DIM, G_C], f32, kind="ExternalOutput")
    osum_d = nc.dram_tensor("osum", [OUT_DIM, G_C], f32, kind="ExternalOutput")

    n_blk = (M_C + BLK - 1) // BLK
    GB = BLK // GS            # groups per full block

    with tile.TileContext(nc) as tc:
        with (
            tc.tile_pool(name="singles", bufs=1) as singles,
            tc.tile_pool(name="xin", bufs=3) as xin,
            tc.tile_pool(name="rsb", bufs=3) as rsb,
            tc.tile_pool(name="tree", bufs=3) as tpool,
            tc.tile_pool(name="psum", bufs=2, space="PSUM") as psum,
        ):
            wt_sb = singles.tile([IN_DIM, OUT_DIM], bf16)
            nc.sync.dma_start(out=wt_sb, in_=wt_d[:, :])
            bsc_sb = singles.tile([128, N_CHUNK], f32)
            nc.sync.dma_start(out=bsc_sb, in_=bsc_d[:, :])

            # prime the ACT spline-table load while the first DMA is in flight
            warm_sb = singles.tile([128, 2], f32)
            nc.vector.memset(warm_sb, 0.0)
            nc.scalar.activation(
                out=warm_sb, in_=warm_sb, func=RELU, bias=0.0, scale=1.0,
            )

            blocks = [
                (i * BLK, min(BLK, M_C - i * BLK)) for i in range(n_blk)
            ]

            for ib, (l0, lb) in enumerate(blocks):
                gb = lb // GS
                g0 = l0 // GS

                xt_sb = xin.tile([IN_DIM, BLK], bf16, tag="xt")
                nc.sync.dma_start(out=xt_sb[:, :lb], in_=xt_d[:, l0 : l0 + lb])

                # per-block relu'd activations, all 4 chunks: [128, 4, BLK]
                r_sb = rsb.tile([128, N_CHUNK, BLK], bf16, tag="r")

                n_wave = (lb + 511) // 512
                for c in range(N_CHUNK):
                    enc_ps = psum.tile([128, BLK], f32, tag="enc")
                    for w in range(n_wave):
                        w0 = w * 512
                        lw = min(512, lb - w0)
                        nc.tensor.matmul(
                            enc_ps[:, w0 : w0 + lw],
                            wt_sb[:, c * 128 : (c + 1) * 128],
                            xt_sb[:, w0 : w0 + lw],
                            start=True, stop=True,
                        )
                    nc.scalar.activation(
                        out=r_sb[:, c, :lb], in_=enc_ps[:, :lb],
                        func=RELU, bias=bsc_sb[:, c : c + 1], scale=1.0,
                    )

                # Block-fused pooling trees. 4D APs run ~4x slower on DVE,
                # so flatten (chunk, group) into ONE dim — exact for full
                # blocks (gb == GB makes the strides uniform); the last
                # partial block falls back to per-chunk 3D trees.
                eng_s = (
                    nc.gpsimd if (ib * GPS_NUM) % GPS_DEN < GPS_NUM
                    else nc.vector
                )
                psum_o = tpool.tile([128, N_CHUNK, GB], f32, tag="psum_o")
                pmax_o = tpool.tile([128, N_CHUNK, GB], f32, tag="pmax_o")
                t1s = tpool.tile([128, N_CHUNK * GB, 8], bf16, tag="t1s")
                t2s = tpool.tile([128, N_CHUNK * GB, 4], bf16, tag="t2s")
                t3s = tpool.tile([128, N_CHUNK * GB, 2], bf16, tag="t3s")
                t1m = tpool.tile([128, N_CHUNK * GB, 8], bf16, tag="t1m")
                t2m = tpool.tile([128, N_CHUNK * GB, 4], bf16, tag="t2m")
                t3m = tpool.tile([128, N_CHUNK * GB, 2], bf16, tag="t3m")

                if gb == GB:
                    views = [(
                        r_sb.rearrange("p c (g s) -> p (c g) s", s=GS),
                        t1s, t2s, t3s,
                        psum_o.rearrange("p c g -> p (c g)"),
                        t1m, t2m, t3m,
                        pmax_o.rearrange("p c g -> p (c g)"),
                    )]
                else:
                    views = []
                    for c in range(N_CHUNK):
                        cs = slice(c * GB, c * GB + gb)
                        views.append((
                            r_sb[:, c, :lb].rearrange(
                                "p (g s) -> p g s", s=GS),
                            t1s[:, cs, :], t2s[:, cs, :], t3s[:, cs, :],
                            psum_o[:, c, :gb],
                            t1m[:, cs, :], t2m[:, cs, :], t3m[:, cs, :],
                            pmax_o[:, c, :gb],
                        ))

                # issue order interleaves the two independent chains so
                # back-to-back DVE instructions never depend on each other
                # (a chained op stalls ~0.5us on its producer's write-ack)
                for rv, u1, u2, u3, so, v1, v2, v3, mo in views:
                    nc.vector.tensor_tensor(
                        out=v1, in0=rv[:, :, 0:8], in1=rv[:, :, 8:16], op=MAX)
                    eng_s.tensor_tensor(
                        out=u1, in0=rv[:, :, 0:8], in1=rv[:, :, 8:16], op=ADD)
                    nc.vector.tensor_tensor(
                        out=v2, in0=v1[:, :, 0:4], in1=v1[:, :, 4:8], op=MAX)
                    eng_s.tensor_tensor(
                        out=u2, in0=u1[:, :, 0:4], in1=u1[:, :, 4:8], op=ADD)
                    nc.vector.tensor_tensor(
                        out=v3, in0=v2[:, :, 0:2], in1=v2[:, :, 2:4], op=MAX)
                    eng_s.tensor_tensor(
                        out=u3, in0=u2[:, :, 0:2], in1=u2[:, :, 2:4], op=ADD)
                    nc.vector.tensor_tensor(
                        out=mo, in0=v3[:, :, 0], in1=v3[:, :, 1], op=MAX)
                    eng_s.tensor_tensor(
                        out=so, in0=u3[:, :, 0], in1=u3[:, :, 1], op=ADD)

                # stream this block's pooled groups straight out; one DMA
                # per output covers all 4 chunks (regular row stride)
                omax_v = omax_d.rearrange("(c p) g -> p c g", p=128)
                osum_v = osum_d.rearrange("(c p) g -> p c g", p=128)
                nc.sync.dma_start(
                    out=omax_v[:, :, g0 : g0 + gb], in_=pmax_o[:, :, :gb])
                nc.sync.dma_start(
                    out=osum_v[:, :, g0 : g0 + gb], in_=psum_o[:, :, :gb])

    nc.compile()
    return nc


def _get_nc(mode: str) -> bass.Bass:
    if mode not in _compiled:
        _compiled[mode] = _build(mode)
    return _compiled[mode]


def _host_prep(lane_encoding, W, b, mode: str):
    """Returns the per-core in_maps."""
    bf = ml_dtypes.bfloat16
    xT = np.ascontiguousarray(lane_encoding.T).astype(bf)   # [128, M] bf16
    wT = np.ascontiguousarray(W.T).astype(bf)               # [128, 512] bf16
    bsc = np.ascontiguousarray(b.reshape(N_CHUNK, 128).T.astype(np.float32))

    in_maps = []
    for c in range(N_CORES):
        sl = slice(c * M_C, (c + 1) * M_C)
        in_maps.append({
            "xt": np.ascontiguousarray(xT[:, sl]),
            "wt": wT, "bsc": bsc,
        })
    return in_maps


def _run(lane_encoding, W, b, mode: str = "fused", trace: bool = False):
    nc = _get_nc(mode)
    in_maps = _host_prep(lane_encoding, W, b, mode)
    try:
        res = run_bass_kernel_spmd(
            nc, in_maps, core_ids=list(range(N_CORES)), trace=trace
        )
    except Exception:
        # transient NRT_EXEC_UNIT_UNRECOVERABLE wedges have been observed;
        # a single retry usually succeeds
        res = run_bass_kernel_spmd(
            nc, in_maps, core_ids=list(range(N_CORES)), trace=trace
        )
    out = np.empty((N_OBS, 2 * OUT_DIM), dtype=np.float32)
    inv_gs = np.float32(1.0 / GS)
    for c in range(N_CORES):
        gsl = slice(c * G_C, (c + 1) * G_C)
        out[gsl, :OUT_DIM] = res.results[c]["omax"].T
        out[gsl, OUT_DIM:] = res.results[c]["osum"].T * inv_gs
    return out, res


MODE = "fused"


def kernel(obs_encoding, lane_encoding, same_obs_mask, W, b):
    out, _ = _run(
        np.asarray(lane_encoding, dtype=np.float32),
        np.asarray(W, dtype=np.float32),
        np.asarray(b, dtype=np.float32),
        MODE,
    )
    return out


# revision 12
# speedup vs baseline: 1.8417x; 1.1596x over previous
"""Trainium2 Bass kernel for AttentionalAggregation-style GNN pooling.

reference math:
    enc  = relu(lane_encoding @ W.T + b)            # [M=400000, 512]
    maxp = segment_max(enc, seg)                    # [N=25000, 512], 16 lanes/group
    avgp = segment_mean(enc, seg)                   # [N=25000, 512]
    out  = concat([maxp, avgp], axis=1)             # [N, 2*512]

Strategy (8 NeuronCores, data-parallel over lanes; each core owns whole groups):
  - Host pre-transposes x -> XT [128, M] bf16 so the contraction dim is the
    SBUF partition dim for both matmul operands. Single-pass bf16 matmul
    (~4e-3 rel err, gate is 2e-2) -> PE time is 1/3 of a compensated bf16x3.
  - PSUM is consumed by relu(u+b) -> r, a per-BLOCK [128, 4, 2048] bf16
    tile. Mostly on ACT (fused relu+bias on the PSUM read); a knob moves
    some chunks to DVE tensor_scalar (add-bias, max-0) to balance engines.
  - Pooling runs as a radix-2 pairwise tree in bf16 on DVE, fused across
    all 4 outdim chunks per block via a flattened (chunk group) dim
    (4D APs run ~4x slower on DVE; the flattened 3D form hits the 2x_1p
    fast path: ~0.56 ns/output).
  - Only tree levels 1-2 run on device. The 4-wide partials (t2) stream
    to HBM in bf16 and the HOST does the final 4->1 sum/max: trades ~66us
    of critical-path DVE time for spare DMA bandwidth, and strictly
    reduces rounding (fewer bf16 additions on device).
  - gpsimd is OFF by default: its tensor_tensor co-streaming the same
    SBUF tiles collapses DVE's 2x mode (~4x slowdown measured), a net
    loss. (It also cannot do max at all.)
  - Sum pool is divided by 16 on the host; max pool is exact (relu is
    monotone, bias applied before pooling).
"""
import sys

sys.path.insert(0, "/opt/trn_rl_repo")

import numpy as np
import ml_dtypes

import concourse.bass as bass
import concourse.bacc as bacc
import concourse.tile as tile
from concourse import mybir
from concourse.bass_utils import run_bass_kernel_spmd

N_CORES = 8
IN_DIM = 128
OUT_DIM = 512
N_OBS = 25000
M_LANES = 400000
GS = 16                       # lanes per group
M_C = M_LANES // N_CORES      # 50000 lanes per core
G_C = N_OBS // N_CORES        # 3125 groups per core
N_CHUNK = OUT_DIM // 128      # 4 outdim chunks
BLK = 2048                    # lanes per DMA/compute block (4 psum banks)
BLK0 = 512                    # small first block to prime the pipeline

# --- load-balance knobs (tuned against the ntff profile) -------------------
GPS_NUM = 0       # blocks whose SUM tree goes to gpsimd (keep 0: see above)
GPS_DEN = 25
DVE_RELU_NTH = 16  # every Nth chunk-relu runs on DVE instead of ACT (0=off)

_compiled = {}


def _build(mode: str) -> bass.Bass:
    nc = bacc.Bacc(None, target_bir_lowering=False)
    f32 = mybir.dt.float32
    bf16 = mybir.dt.bfloat16
    ADD = mybir.AluOpType.add
    MAX = mybir.AluOpType.max
    RELU = mybir.ActivationFunctionType.Relu

    xt_d = nc.dram_tensor("xt", [IN_DIM, M_C], bf16, kind="ExternalInput")
    wt_d = nc.dram_tensor("wt", [IN_DIM, OUT_DIM], bf16, kind="ExternalInput")
    bsc_d = nc.dram_tensor("bsc", [128, N_CHUNK], f32, kind="ExternalInput")
    # 4-wide pooled partials; host finishes the last two tree levels
    omax_d = nc.dram_tensor(
        "omax4", [OUT_DIM, G_C * 4], bf16, kind="ExternalOutput")
    osum_d = nc.dram_tensor(
        "osum4", [OUT_DIM, G_C * 4], bf16, kind="ExternalOutput")
    omax_v = omax_d.rearrange("(c p) (g s) -> p c g s", p=128, s=4)
    osum_v = osum_d.rearrange("(c p) (g s) -> p c g s", p=128, s=4)

    GB = BLK // GS            # groups per full block

    with tile.TileContext(nc) as tc:
        with (
            tc.tile_pool(name="singles", bufs=1) as singles,
            tc.tile_pool(name="xin", bufs=4) as xin,
            tc.tile_pool(name="rsb", bufs=3) as rsb,
            tc.tile_pool(name="tree", bufs=3) as tpool,
            tc.tile_pool(name="psum", bufs=2, space="PSUM") as psum,
        ):
            wt_sb = singles.tile([IN_DIM, OUT_DIM], bf16)
            nc.sync.dma_start(out=wt_sb, in_=wt_d[:, :])
            bsc_sb = singles.tile([128, N_CHUNK], f32)
            nc.sync.dma_start(out=bsc_sb, in_=bsc_d[:, :])

            # prime the ACT spline-table load while the first DMA is in flight
            warm_sb = singles.tile([128, 2], f32)
            nc.vector.memset(warm_sb, 0.0)
            nc.scalar.activation(
                out=warm_sb, in_=warm_sb, func=RELU, bias=0.0, scale=1.0,
            )

            blocks = [(0, BLK0)]
            while blocks[-1][0] + blocks[-1][1] < M_C:
                s = blocks[-1][0] + blocks[-1][1]
                blocks.append((s, min(BLK, M_C - s)))

            chunk_idx = 0
            for ib, (l0, lb) in enumerate(blocks):
                gb = lb // GS
                g0 = l0 // GS

                xt_sb = xin.tile([IN_DIM, BLK], bf16, tag="xt")
                nc.sync.dma_start(out=xt_sb[:, :lb], in_=xt_d[:, l0 : l0 + lb])

                # per-block relu'd activations, all 4 chunks: [128, 4, BLK]
                r_sb = rsb.tile([128, N_CHUNK, BLK], bf16, tag="r")

                n_wave = (lb + 511) // 512
                for c in range(N_CHUNK):
                    enc_ps = psum.tile([128, BLK], f32, tag="enc")
                    for w in range(n_wave):
                        w0 = w * 512
                        lw = min(512, lb - w0)
                        nc.tensor.matmul(
                            enc_ps[:, w0 : w0 + lw],
                            wt_sb[:, c * 128 : (c + 1) * 128],
                            xt_sb[:, w0 : w0 + lw],
                            start=True, stop=True,
                        )
                    chunk_idx += 1
                    if DVE_RELU_NTH and chunk_idx % DVE_RELU_NTH == 0:
                        # balance: run this chunk's relu on DVE instead
                        nc.vector.tensor_scalar(
                            out=r_sb[:, c, :lb], in0=enc_ps[:, :lb],
                            scalar1=bsc_sb[:, c : c + 1], scalar2=0.0,
                            op0=ADD, op1=MAX,
                        )
                    else:
                        nc.scalar.activation(
                            out=r_sb[:, c, :lb], in_=enc_ps[:, :lb],
                            func=RELU, bias=bsc_sb[:, c : c + 1], scale=1.0,
                        )

                # Block-fused pooling trees, levels 1-2 only. 4D APs run ~4x
                # slower on DVE, so flatten (chunk, group) into ONE dim —
                # exact for full blocks; partial blocks go per-chunk.
                eng_s = (
                    nc.gpsimd if GPS_NUM and (ib * GPS_NUM) % GPS_DEN < GPS_NUM
                    else nc.vector
                )
                t1s = tpool.tile([128, N_CHUNK * GB, 8], bf16, tag="t1s")
                t2s = tpool.tile([128, N_CHUNK * GB, 4], bf16, tag="t2s")
                t1m = tpool.tile([128, N_CHUNK * GB, 8], bf16, tag="t1m")
                t2m = tpool.tile([128, N_CHUNK * GB, 4], bf16, tag="t2m")

                if gb == GB:
                    views = [(
                        r_sb.rearrange("p c (g s) -> p (c g) s", s=GS),
                        t1s, t2s, t1m, t2m,
                    )]
                else:
                    views = []
                    for c in range(N_CHUNK):
                        cs = slice(c * GB, c * GB + gb)
                        views.append((
                            r_sb[:, c, :lb].rearrange(
                                "p (g s) -> p g s", s=GS),
                            t1s[:, cs, :], t2s[:, cs, :],
                            t1m[:, cs, :], t2m[:, cs, :],
                        ))

                for rv, u1, u2, v1, v2 in views:
                    nc.vector.tensor_tensor(
                        out=v1, in0=rv[:, :, 0:8], in1=rv[:, :, 8:16], op=MAX)
                    eng_s.tensor_tensor(
                        out=u1, in0=rv[:, :, 0:8], in1=rv[:, :, 8:16], op=ADD)
                    nc.vector.tensor_tensor(
                        out=v2, in0=v1[:, :, 0:4], in1=v1[:, :, 4:8], op=MAX)
                    eng_s.tensor_tensor(
                        out=u2, in0=u1[:, :, 0:4], in1=u1[:, :, 4:8], op=ADD)

                # stream this block's 4-wide partials straight out; one DMA
                # per output covers all 4 chunks (regular row stride)
                t2s_v = t2s.rearrange("p (c g) s -> p c g s", g=GB)
                t2m_v = t2m.rearrange("p (c g) s -> p c g s", g=GB)
                nc.sync.dma_start(
                    out=omax_v[:, :, g0 : g0 + gb, :],
                    in_=t2m_v[:, :, :gb, :])
                nc.sync.dma_start(
                    out=osum_v[:, :, g0 : g0 + gb, :],
                    in_=t2s_v[:, :, :gb, :])

    nc.compile()
    return nc


def _get_nc(mode: str) -> bass.Bass:
    if mode not in _compiled:
        _compiled[mode] = _build(mode)
    return _compiled[mode]


def _host_prep(lane_encoding, W, b, mode: str):
    """Returns the per-core in_maps."""
    bf = ml_dtypes.bfloat16
    xT = np.ascontiguousarray(lane_encoding.T).astype(bf)   # [128, M] bf16
    wT = np.ascontiguousarray(W.T).astype(bf)               # [128, 512] bf16
    bsc = np.ascontiguousarray(b.reshape(N_CHUNK, 128).T.astype(np.float32))

    in_maps = []
    for c in range(N_CORES):
        sl = slice(c * M_C, (c + 1) * M_C)
        in_maps.append({
            "xt": np.ascontiguousarray(xT[:, sl]),
            "wt": wT, "bsc": bsc,
        })
    return in_maps


def _run(lane_encoding, W, b, mode: str = "fused", trace: bool = False):
    nc = _get_nc(mode)
    in_maps = _host_prep(lane_encoding, W, b, mode)
    try:
        res = run_bass_kernel_spmd(
            nc, in_maps, core_ids=list(range(N_CORES)), trace=trace
        )
    except Exception:
        # transient NRT_EXEC_UNIT_UNRECOVERABLE wedges have been observed;
        # a single retry usually succeeds
        res = run_bass_kernel_spmd(
            nc, in_maps, core_ids=list(range(N_CORES)), trace=trace
        )
    out = np.empty((N_OBS, 2 * OUT_DIM), dtype=np.float32)
    inv_gs = np.float32(1.0 / GS)
    for c in range(N_CORES):
        gsl = slice(c * G_C, (c + 1) * G_C)
        m4 = res.results[c]["omax4"].astype(np.float32).reshape(OUT_DIM, G_C, 4)
        s4 = res.results[c]["osum4"].astype(np.float32).reshape(OUT_DIM, G_C, 4)
        out[gsl, :OUT_DIM] = m4.max(axis=2).T
        out[gsl, OUT_DIM:] = s4.sum(axis=2).T * inv_gs
    return out, res


MODE = "fused"


def kernel(obs_encoding, lane_encoding, same_obs_mask, W, b):
    out, _ = _run(
        np.asarray(lane_encoding, dtype=np.float32),
        np.asarray(W, dtype=np.float32),
        np.asarray(b, dtype=np.float32),
        MODE,
    )
    return out


# revision 13
# speedup vs baseline: 1.8503x; 1.0047x over previous
"""Trainium2 Bass kernel for AttentionalAggregation-style GNN pooling.

reference math:
    enc  = relu(lane_encoding @ W.T + b)            # [M=400000, 512]
    maxp = segment_max(enc, seg)                    # [N=25000, 512], 16 lanes/group
    avgp = segment_mean(enc, seg)                   # [N=25000, 512]
    out  = concat([maxp, avgp], axis=1)             # [N, 2*512]

Strategy (8 NeuronCores, data-parallel over lanes; each core owns whole groups):
  - Host pre-transposes x -> XT [128, M] bf16 so the contraction dim is the
    SBUF partition dim for both matmul operands. Single-pass bf16 matmul
    (~4e-3 rel err, gate is 2e-2) -> PE time is 1/3 of a compensated bf16x3.
  - PSUM is consumed by relu(u+b) -> r, a per-BLOCK [128, 4, 2048] bf16
    tile. Mostly on ACT (fused relu+bias on the PSUM read); a knob moves
    some chunks to DVE tensor_scalar (add-bias, max-0) to balance engines.
  - Pooling runs as a radix-2 pairwise tree in bf16 on DVE, fused across
    all 4 outdim chunks per block via a flattened (chunk group) dim
    (4D APs run ~4x slower on DVE; the flattened 3D form hits the 2x_1p
    fast path: ~0.56 ns/output).
  - Only tree levels 1-2 run on device. The 4-wide partials (t2) stream
    to HBM in bf16 and the HOST does the final 4->1 sum/max: trades ~66us
    of critical-path DVE time for spare DMA bandwidth, and strictly
    reduces rounding (fewer bf16 additions on device).
  - gpsimd is OFF by default: its tensor_tensor co-streaming the same
    SBUF tiles collapses DVE's 2x mode (~4x slowdown measured), a net
    loss. (It also cannot do max at all.)
  - Sum pool is divided by 16 on the host; max pool is exact (relu is
    monotone, bias applied before pooling).
"""
import sys

sys.path.insert(0, "/opt/trn_rl_repo")

import numpy as np
import ml_dtypes

import concourse.bass as bass
import concourse.bacc as bacc
import concourse.tile as tile
from concourse import mybir
from concourse.bass_utils import run_bass_kernel_spmd

N_CORES = 8
IN_DIM = 128
OUT_DIM = 512
N_OBS = 25000
M_LANES = 400000
GS = 16                       # lanes per group
M_C = M_LANES // N_CORES      # 50000 lanes per core
G_C = N_OBS // N_CORES        # 3125 groups per core
N_CHUNK = OUT_DIM // 128      # 4 outdim chunks
BLK = 2048                    # lanes per DMA/compute block (4 psum banks)
BLK0 = 512                    # small first block to prime the pipeline

# --- load-balance knobs (tuned against the ntff profile) -------------------
GPS_NUM = 0       # blocks whose SUM tree goes to gpsimd (keep 0: see above)
GPS_DEN = 25
DVE_RELU_NTH = 16  # every Nth chunk-relu runs on DVE instead of ACT (0=off)

_compiled = {}


def _build(mode: str) -> bass.Bass:
    nc = bacc.Bacc(None, target_bir_lowering=False)
    f32 = mybir.dt.float32
    bf16 = mybir.dt.bfloat16
    ADD = mybir.AluOpType.add
    MAX = mybir.AluOpType.max
    RELU = mybir.ActivationFunctionType.Relu

    xt_d = nc.dram_tensor("xt", [IN_DIM, M_C], bf16, kind="ExternalInput")
    wt_d = nc.dram_tensor("wt", [IN_DIM, OUT_DIM], bf16, kind="ExternalInput")
    bsc_d = nc.dram_tensor("bsc", [128, N_CHUNK], f32, kind="ExternalInput")
    # 4-wide pooled partials; host finishes the last two tree levels
    omax_d = nc.dram_tensor(
        "omax4", [OUT_DIM, G_C * 4], bf16, kind="ExternalOutput")
    osum_d = nc.dram_tensor(
        "osum4", [OUT_DIM, G_C * 4], bf16, kind="ExternalOutput")
    omax_v = omax_d.rearrange("(c p) (g s) -> p c g s", p=128, s=4)
    osum_v = osum_d.rearrange("(c p) (g s) -> p c g s", p=128, s=4)

    GB = BLK // GS            # groups per full block

    with tile.TileContext(nc) as tc:
        with (
            tc.tile_pool(name="singles", bufs=1) as singles,
            tc.tile_pool(name="xin", bufs=4) as xin,
            tc.tile_pool(name="rsb", bufs=4) as rsb,
            tc.tile_pool(name="tree", bufs=4) as tpool,
            tc.tile_pool(name="psum", bufs=2, space="PSUM") as psum,
        ):
            wt_sb = singles.tile([IN_DIM, OUT_DIM], bf16)
            nc.sync.dma_start(out=wt_sb, in_=wt_d[:, :])
            bsc_sb = singles.tile([128, N_CHUNK], f32)
            nc.sync.dma_start(out=bsc_sb, in_=bsc_d[:, :])

            # prime the ACT spline-table load while the first DMA is in flight
            warm_sb = singles.tile([128, 2], f32)
            nc.vector.memset(warm_sb, 0.0)
            nc.scalar.activation(
                out=warm_sb, in_=warm_sb, func=RELU, bias=0.0, scale=1.0,
            )

            blocks = [(0, BLK0)]
            while blocks[-1][0] + blocks[-1][1] < M_C:
                s = blocks[-1][0] + blocks[-1][1]
                blocks.append((s, min(BLK, M_C - s)))
            # partial blocks first so the kernel tail is a full streamlined
            # block; their per-chunk trees also start DVE work sooner
            blocks.sort(key=lambda b: (b[1] == BLK, b[0]))

            chunk_idx = 0
            for ib, (l0, lb) in enumerate(blocks):
                gb = lb // GS
                g0 = l0 // GS

                xt_sb = xin.tile([IN_DIM, BLK], bf16, tag="xt")
                nc.sync.dma_start(out=xt_sb[:, :lb], in_=xt_d[:, l0 : l0 + lb])

                # per-block relu'd activations, all 4 chunks: [128, 4, BLK]
                r_sb = rsb.tile([128, N_CHUNK, BLK], bf16, tag="r")

                n_wave = (lb + 511) // 512
                for c in range(N_CHUNK):
                    enc_ps = psum.tile([128, BLK], f32, tag="enc")
                    for w in range(n_wave):
                        w0 = w * 512
                        lw = min(512, lb - w0)
                        nc.tensor.matmul(
                            enc_ps[:, w0 : w0 + lw],
                            wt_sb[:, c * 128 : (c + 1) * 128],
                            xt_sb[:, w0 : w0 + lw],
                            start=True, stop=True,
                        )
                    chunk_idx += 1
                    if DVE_RELU_NTH and chunk_idx % DVE_RELU_NTH == 0:
                        # balance: run this chunk's relu on DVE instead
                        nc.vector.tensor_scalar(
                            out=r_sb[:, c, :lb], in0=enc_ps[:, :lb],
                            scalar1=bsc_sb[:, c : c + 1], scalar2=0.0,
                            op0=ADD, op1=MAX,
                        )
                    else:
                        nc.scalar.activation(
                            out=r_sb[:, c, :lb], in_=enc_ps[:, :lb],
                            func=RELU, bias=bsc_sb[:, c : c + 1], scale=1.0,
                        )

                # Block-fused pooling trees, levels 1-2 only. 4D APs run ~4x
                # slower on DVE, so flatten (chunk, group) into ONE dim —
                # exact for full blocks; partial blocks go per-chunk.
                eng_s = (
                    nc.gpsimd if GPS_NUM and (ib * GPS_NUM) % GPS_DEN < GPS_NUM
                    else nc.vector
                )
                t1s = tpool.tile([128, N_CHUNK * GB, 8], bf16, tag="t1s")
                t2s = tpool.tile([128, N_CHUNK * GB, 4], bf16, tag="t2s")
                t1m = tpool.tile([128, N_CHUNK * GB, 8], bf16, tag="t1m")
                t2m = tpool.tile([128, N_CHUNK * GB, 4], bf16, tag="t2m")

                if gb == GB and ib >= 2:
                    views = [(
                        r_sb.rearrange("p c (g s) -> p (c g) s", s=GS),
                        t1s, t2s, t1m, t2m,
                    )]
                else:
                    views = []
                    for c in range(N_CHUNK):
                        cs = slice(c * GB, c * GB + gb)
                        views.append((
                            r_sb[:, c, :lb].rearrange(
                                "p (g s) -> p g s", s=GS),
                            t1s[:, cs, :], t2s[:, cs, :],
                            t1m[:, cs, :], t2m[:, cs, :],
                        ))

                for rv, u1, u2, v1, v2 in views:
                    nc.vector.tensor_tensor(
                        out=v1, in0=rv[:, :, 0:8], in1=rv[:, :, 8:16], op=MAX)
                    eng_s.tensor_tensor(
                        out=u1, in0=rv[:, :, 0:8], in1=rv[:, :, 8:16], op=ADD)
                    nc.vector.tensor_tensor(
                        out=v2, in0=v1[:, :, 0:4], in1=v1[:, :, 4:8], op=MAX)
                    eng_s.tensor_tensor(
                        out=u2, in0=u1[:, :, 0:4], in1=u1[:, :, 4:8], op=ADD)

                # stream this block's 4-wide partials straight out; one DMA
                # per output covers all 4 chunks (regular row stride)
                t2s_v = t2s.rearrange("p (c g) s -> p c g s", g=GB)
                t2m_v = t2m.rearrange("p (c g) s -> p c g s", g=GB)
                nc.sync.dma_start(
                    out=omax_v[:, :, g0 : g0 + gb, :],
                    in_=t2m_v[:, :, :gb, :])
                nc.sync.dma_start(
                    out=osum_v[:, :, g0 : g0 + gb, :],
                    in_=t2s_v[:, :, :gb, :])

    nc.compile()
    return nc


def _get_nc(mode: str) -> bass.Bass:
    if mode not in _compiled:
        _compiled[mode] = _build(mode)
    return _compiled[mode]


def _host_prep(lane_encoding, W, b, mode: str):
    """Returns the per-core in_maps."""
    bf = ml_dtypes.bfloat16
    xT = np.ascontiguousarray(lane_encoding.T).astype(bf)   # [128, M] bf16
    wT = np.ascontiguousarray(W.T).astype(bf)               # [128, 512] bf16
    bsc = np.ascontiguousarray(b.reshape(N_CHUNK, 128).T.astype(np.float32))

    in_maps = []
    for c in range(N_CORES):
        sl = slice(c * M_C, (c + 1) * M_C)
        in_maps.append({
            "xt": np.ascontiguousarray(xT[:, sl]),
            "wt": wT, "bsc": bsc,
        })
    return in_maps


def _run(lane_encoding, W, b, mode: str = "fused", trace: bool = False):
    nc = _get_nc(mode)
    in_maps = _host_prep(lane_encoding, W, b, mode)
    try:
        res = run_bass_kernel_spmd(
            nc, in_maps, core_ids=list(range(N_CORES)), trace=trace
        )
    except Exception:
        # transient NRT_EXEC_UNIT_UNRECOVERABLE wedges have been observed;
        # a single retry usually succeeds
        res = run_bass_kernel_spmd(
            nc, in_maps, core_ids=list(range(N_CORES)), trace=trace
        )
    out = np.empty((N_OBS, 2 * OUT_DIM), dtype=np.float32)
    inv_gs = np.float32(1.0 / GS)
    for c in range(N_CORES):
        gsl = slice(c * G_C, (c + 1) * G_C)
        m4 = res.results[c]["omax4"].astype(np.float32).reshape(OUT_DIM, G_C, 4)
        s4 = res.results[c]["osum4"].astype(np.float32).reshape(OUT_DIM, G_C, 4)
        out[gsl, :OUT_DIM] = m4.max(axis=2).T
        out[gsl, OUT_DIM:] = s4.sum(axis=2).T * inv_gs
    return out, res


MODE = "fused"


def kernel(obs_encoding, lane_encoding, same_obs_mask, W, b):
    out, _ = _run(
        np.asarray(lane_encoding, dtype=np.float32),
        np.asarray(W, dtype=np.float32),
        np.asarray(b, dtype=np.float32),
        MODE,
    )
    return out


# revision 14
# speedup vs baseline: 1.8740x; 1.0128x over previous
"""Trainium2 Bass kernel for AttentionalAggregation-style GNN pooling.

reference math:
    enc  = relu(lane_encoding @ W.T + b)            # [M=400000, 512]
    maxp = segment_max(enc, seg)                    # [N=25000, 512], 16 lanes/group
    avgp = segment_mean(enc, seg)                   # [N=25000, 512]
    out  = concat([maxp, avgp], axis=1)             # [N, 2*512]

Strategy (8 NeuronCores, data-parallel over lanes; each core owns whole groups):
  - Host pre-transposes x -> XT [128, M] bf16 so the contraction dim is the
    SBUF partition dim for both matmul operands. Single-pass bf16 matmul
    (~4e-3 rel err, gate is 2e-2) -> PE time is 1/3 of a compensated bf16x3.
  - PSUM is consumed by relu(u+b) -> r, a per-BLOCK [128, 4, 2048] bf16
    tile. Mostly on ACT (fused relu+bias on the PSUM read); a knob moves
    some chunks to DVE tensor_scalar (add-bias, max-0) to balance engines.
  - Pooling runs as a radix-2 pairwise tree in bf16 on DVE, fused across
    all 4 outdim chunks per block via a flattened (chunk group) dim
    (4D APs run ~4x slower on DVE; the flattened 3D form hits the 2x_1p
    fast path: ~0.56 ns/output).
  - Only tree levels 1-2 run on device. The 4-wide partials (t2) stream
    to HBM in bf16 and the HOST does the final 4->1 sum/max: trades ~66us
    of critical-path DVE time for spare DMA bandwidth, and strictly
    reduces rounding (fewer bf16 additions on device).
  - gpsimd is OFF by default: its tensor_tensor co-streaming the same
    SBUF tiles collapses DVE's 2x mode (~4x slowdown measured), a net
    loss. (It also cannot do max at all.)
  - Sum pool is divided by 16 on the host; max pool is exact (relu is
    monotone, bias applied before pooling).
"""
import sys

sys.path.insert(0, "/opt/trn_rl_repo")

import numpy as np
import ml_dtypes

import concourse.bass as bass
import concourse.bacc as bacc
import concourse.tile as tile
from concourse import mybir
from concourse.bass_utils import run_bass_kernel_spmd

N_CORES = 8
IN_DIM = 128
OUT_DIM = 512
N_OBS = 25000
M_LANES = 400000
GS = 16                       # lanes per group
M_C = M_LANES // N_CORES      # 50000 lanes per core
G_C = N_OBS // N_CORES        # 3125 groups per core
N_CHUNK = OUT_DIM // 128      # 4 outdim chunks
BLK = 2048                    # lanes per DMA/compute block (4 psum banks)
BLK0 = 512                    # small first block to prime the pipeline

# --- load-balance knobs (tuned against the ntff profile) -------------------
GPS_NUM = 0       # blocks whose SUM tree goes to gpsimd (keep 0: see above)
GPS_DEN = 25
DVE_RELU_NTH = 16  # every Nth chunk-relu runs on DVE instead of ACT (0=off)

_compiled = {}


def _build(mode: str) -> bass.Bass:
    nc = bacc.Bacc(None, target_bir_lowering=False)
    f32 = mybir.dt.float32
    bf16 = mybir.dt.bfloat16
    ADD = mybir.AluOpType.add
    MAX = mybir.AluOpType.max
    RELU = mybir.ActivationFunctionType.Relu

    xt_d = nc.dram_tensor("xt", [IN_DIM, M_C], bf16, kind="ExternalInput")
    wt_d = nc.dram_tensor("wt", [IN_DIM, OUT_DIM], bf16, kind="ExternalInput")
    bsc_d = nc.dram_tensor("bsc", [128, N_CHUNK], f32, kind="ExternalInput")
    # 4-wide pooled partials; host finishes the last two tree levels
    omax_d = nc.dram_tensor(
        "omax4", [OUT_DIM, G_C * 4], bf16, kind="ExternalOutput")
    osum_d = nc.dram_tensor(
        "osum4", [OUT_DIM, G_C * 4], bf16, kind="ExternalOutput")
    omax_v = omax_d.rearrange("(c p) (g s) -> p c g s", p=128, s=4)
    osum_v = osum_d.rearrange("(c p) (g s) -> p c g s", p=128, s=4)

    GB = BLK // GS            # groups per full block

    with tile.TileContext(nc) as tc:
        with (
            tc.tile_pool(name="singles", bufs=1) as singles,
            tc.tile_pool(name="xin", bufs=4) as xin,
            tc.tile_pool(name="rsb", bufs=4) as rsb,
            tc.tile_pool(name="tree", bufs=4) as tpool,
            tc.tile_pool(name="psum", bufs=2, space="PSUM") as psum,
        ):
            wt_sb = singles.tile([IN_DIM, OUT_DIM], bf16)
            nc.sync.dma_start(out=wt_sb, in_=wt_d[:, :])
            bsc_sb = singles.tile([128, N_CHUNK], f32)
            nc.sync.dma_start(out=bsc_sb, in_=bsc_d[:, :])

            # prime the ACT spline-table load while the first DMA is in flight
            warm_sb = singles.tile([128, 2], f32)
            nc.vector.memset(warm_sb, 0.0)
            nc.scalar.activation(
                out=warm_sb, in_=warm_sb, func=RELU, bias=0.0, scale=1.0,
            )

            blocks = [(0, BLK0)]
            while blocks[-1][0] + blocks[-1][1] < M_C:
                s = blocks[-1][0] + blocks[-1][1]
                blocks.append((s, min(BLK, M_C - s)))

            chunk_idx = 0
            for ib, (l0, lb) in enumerate(blocks):
                gb = lb // GS
                g0 = l0 // GS

                xt_sb = xin.tile([IN_DIM, BLK], bf16, tag="xt")
                nc.sync.dma_start(out=xt_sb[:, :lb], in_=xt_d[:, l0 : l0 + lb])

                # per-block relu'd activations, all 4 chunks: [128, 4, BLK]
                r_sb = rsb.tile([128, N_CHUNK, BLK], bf16, tag="r")

                n_wave = (lb + 511) // 512
                for c in range(N_CHUNK):
                    enc_ps = psum.tile([128, BLK], f32, tag="enc")
                    for w in range(n_wave):
                        w0 = w * 512
                        lw = min(512, lb - w0)
                        nc.tensor.matmul(
                            enc_ps[:, w0 : w0 + lw],
                            wt_sb[:, c * 128 : (c + 1) * 128],
                            xt_sb[:, w0 : w0 + lw],
                            start=True, stop=True,
                        )
                    chunk_idx += 1
                    if DVE_RELU_NTH and chunk_idx % DVE_RELU_NTH == 0:
                        # balance: run this chunk's relu on DVE instead
                        nc.vector.tensor_scalar(
                            out=r_sb[:, c, :lb], in0=enc_ps[:, :lb],
                            scalar1=bsc_sb[:, c : c + 1], scalar2=0.0,
                            op0=ADD, op1=MAX,
                        )
                    else:
                        nc.scalar.activation(
                            out=r_sb[:, c, :lb], in_=enc_ps[:, :lb],
                            func=RELU, bias=bsc_sb[:, c : c + 1], scale=1.0,
                        )

                # Block-fused pooling trees, levels 1-2 only. 4D APs run ~4x
                # slower on DVE, so flatten (chunk, group) into ONE dim —
                # exact for full blocks; partial blocks go per-chunk.
                eng_s = (
                    nc.gpsimd if GPS_NUM and (ib * GPS_NUM) % GPS_DEN < GPS_NUM
                    else nc.vector
                )
                t1s = tpool.tile([128, N_CHUNK * GB, 8], bf16, tag="t1s")
                t2s = tpool.tile([128, N_CHUNK * GB, 4], bf16, tag="t2s")
                t1m = tpool.tile([128, N_CHUNK * GB, 8], bf16, tag="t1m")
                t2m = tpool.tile([128, N_CHUNK * GB, 4], bf16, tag="t2m")

                if gb == GB and ib >= 3:
                    views = [(
                        r_sb.rearrange("p c (g s) -> p (c g) s", s=GS),
                        t1s, t2s, t1m, t2m,
                    )]
                else:
                    views = []
                    for c in range(N_CHUNK):
                        cs = slice(c * GB, c * GB + gb)
                        views.append((
                            r_sb[:, c, :lb].rearrange(
                                "p (g s) -> p g s", s=GS),
                            t1s[:, cs, :], t2s[:, cs, :],
                            t1m[:, cs, :], t2m[:, cs, :],
                        ))

                for rv, u1, u2, v1, v2 in views:
                    nc.vector.tensor_tensor(
                        out=v1, in0=rv[:, :, 0:8], in1=rv[:, :, 8:16], op=MAX)
                    eng_s.tensor_tensor(
                        out=u1, in0=rv[:, :, 0:8], in1=rv[:, :, 8:16], op=ADD)
                    nc.vector.tensor_tensor(
                        out=v2, in0=v1[:, :, 0:4], in1=v1[:, :, 4:8], op=MAX)
                    eng_s.tensor_tensor(
                        out=u2, in0=u1[:, :, 0:4], in1=u1[:, :, 4:8], op=ADD)

                # stream this block's 4-wide partials straight out; one DMA
                # per output covers all 4 chunks (regular row stride)
                t2s_v = t2s.rearrange("p (c g) s -> p c g s", g=GB)
                t2m_v = t2m.rearrange("p (c g) s -> p c g s", g=GB)
                nc.sync.dma_start(
                    out=omax_v[:, :, g0 : g0 + gb, :],
                    in_=t2m_v[:, :, :gb, :])
                nc.sync.dma_start(
                    out=osum_v[:, :, g0 : g0 + gb, :],
                    in_=t2s_v[:, :, :gb, :])

    nc.compile()
    return nc


def _get_nc(mode: str) -> bass.Bass:
    if mode not in _compiled:
        _compiled[mode] = _build(mode)
    return _compiled[mode]


def _host_prep(lane_encoding, W, b, mode: str):
    """Returns the per-core in_maps."""
    bf = ml_dtypes.bfloat16
    xT = np.ascontiguousarray(lane_encoding.T).astype(bf)   # [128, M] bf16
    wT = np.ascontiguousarray(W.T).astype(bf)               # [128, 512] bf16
    bsc = np.ascontiguousarray(b.reshape(N_CHUNK, 128).T.astype(np.float32))

    in_maps = []
    for c in range(N_CORES):
        sl = slice(c * M_C, (c + 1) * M_C)
        in_maps.append({
            "xt": np.ascontiguousarray(xT[:, sl]),
            "wt": wT, "bsc": bsc,
        })
    return in_maps


def _run(lane_encoding, W, b, mode: str = "fused", trace: bool = False):
    nc = _get_nc(mode)
    in_maps = _host_prep(lane_encoding, W, b, mode)
    try:
        res = run_bass_kernel_spmd(
            nc, in_maps, core_ids=list(range(N_CORES)), trace=trace
        )
    except Exception:
        # transient NRT_EXEC_UNIT_UNRECOVERABLE wedges have been observed;
        # a single retry usually succeeds
        res = run_bass_kernel_spmd(
            nc, in_maps, core_ids=list(range(N_CORES)), trace=trace
        )
    out = np.empty((N_OBS, 2 * OUT_DIM), dtype=np.float32)
    inv_gs = np.float32(1.0 / GS)
    for c in range(N_CORES):
        gsl = slice(c * G_C, (c + 1) * G_C)
        m4 = res.results[c]["omax4"].astype(np.float32).reshape(OUT_DIM, G_C, 4)
        s4 = res.results[c]["osum4"].astype(np.float32).reshape(OUT_DIM, G_C, 4)
        out[gsl, :OUT_DIM] = m4.max(axis=2).T
        out[gsl, OUT_DIM:] = s4.sum(axis=2).T * inv_gs
    return out, res


MODE = "fused"


def kernel(obs_encoding, lane_encoding, same_obs_mask, W, b):
    out, _ = _run(
        np.asarray(lane_encoding, dtype=np.float32),
        np.asarray(W, dtype=np.float32),
        np.asarray(b, dtype=np.float32),
        MODE,
    )
    return out
